# revision 21
# baseline (speedup 1.0000x reference)
"""Embedding lookup (weight[indices]) on 8 TRN2 NeuronCores.

Table replicated per core (bf16), indices sharded 8 ways.  Per-row
indirect DMA costs ~8.6 ns/row of serial Q7 descriptor generation, and
dma_scatter_add costs ~5.5 ns/token plus a CCE read-modify-write, so the
kernel uses a two-pass gather radix built only from dma_gather
(~2.3 ns/token, the cheapest data-dependent primitive):

  pass A: host sorts the core's tokens by (out_window, table_window)
          into fixed 1024-token groups (int16 window-local indices,
          duplicate-row pads); windowed dma_gathers fill SBUF tiles in
          stream order; contiguous HWDGE stores write the tiles to an
          HBM temp in partition-major layout (32 KB descriptors).
  pass B: each 32768-row temp segment holds exactly one out-window's
          tokens, so an int16 rank re-gathers them in original order;
          contiguous stores write a partition-major out buffer that the
          host transposes back (static, index-independent reshape).

Group capacity is fixed, so the instruction structure depends only on
shapes: one compiled NEFF serves all cores and calls.  bf16 halves HBM
traffic; its rounding (~0.4% rel err) is well inside the 2e-2 gate.
"""

import numpy as np
import ml_dtypes

P = 128
D = 128
NUM_EMB = 1_000_000
N_CORES = 8

TW_ROWS = 32768     # table window (int16 gather index range)
OW_ROWS = 25600     # out rows per segment (per_core / 4)
CAP = 1024          # fixed tokens per (ow, tw) group; avg ~826
GROUPS_PER_SEG = 32 # 31 real table windows + 1 all-pad group -> 32768 rows
TILE_TOK = 16384    # tokens per SBUF tile (16 groups)
DT_NAME = "bfloat16"

_CACHE = {}


class Plan:
    def __init__(self, num_emb=NUM_EMB, d=D, per_core=102400):
        self.num_emb, self.d, self.per_core = num_emb, d, per_core
        self.n_tw = -(-num_emb // TW_ROWS)           # 31
        assert self.n_tw < GROUPS_PER_SEG
        self.n_ow = per_core // OW_ROWS              # 4
        assert per_core == self.n_ow * OW_ROWS and OW_ROWS % P == 0
        self.seg_tok = GROUPS_PER_SEG * CAP          # 32768 temp rows/segment
        self.n_groups = self.n_ow * GROUPS_PER_SEG   # 128 (incl pad groups)
        self.groups_per_tile = TILE_TOK // CAP       # 16
        self.n_tiles_a = self.n_groups // self.groups_per_tile  # 8
        self.temp_rows = self.n_groups * CAP         # 131072
        # pass A static structure: per tile, gathers (tok_off, tw)
        self.tiles_a = []
        for t in range(self.n_tiles_a):
            g0 = t * self.groups_per_tile
            self.tiles_a.append(
                [((g - g0) * CAP, g % GROUPS_PER_SEG) for g in range(g0, g0 + self.groups_per_tile)]
            )
        # pass B: chunks of TILE_TOK out rows; per chunk, gather runs
        # (tok_off, ntok, ow) split at out-window boundaries
        self.chunks_b = []
        a = 0
        while a < per_core:
            n = min(TILE_TOK, per_core - a)
            runs = []
            o = a
            MAXTOK = 1024   # one gather stays within the SWDGE desc ring
                            # (4096-token gathers crash the device; 1024 is
                            # the empirically safe size, same as pass A's CAP)
            while o < a + n:
                ow = o // OW_ROWS
                e = min(a + n, (ow + 1) * OW_ROWS, o + MAXTOK)
                runs.append((o - a, e - o, ow))
                o = e
            self.chunks_b.append((a, n, runs))
            a += n
        self.n_chunks_b = len(self.chunks_b)

    def route(self, idx):
        n = self.per_core
        assert idx.shape == (n,)
        pos = np.arange(n, dtype=np.int64)
        ow = pos // OW_ROWS
        tw = idx // TW_ROWS
        key = ow * GROUPS_PER_SEG + tw               # pad group (tw=31) unused by real tokens
        order = np.argsort(key, kind="stable")
        counts = np.bincount(key[order], minlength=self.n_groups)
        if counts.max() > CAP:
            raise OverflowError(f"group count {counts.max()} > cap {CAP}")
        total = self.n_tiles_a * TILE_TOK
        g_local = np.zeros(total, dtype=np.int16)    # pass A window-local row
        rank = np.zeros(n, dtype=np.int64)           # token -> padded stream slot
        starts = np.concatenate([[0], np.cumsum(counts)[:-1]])
        for g in range(self.n_groups):
            c = counts[g]
            o = g * CAP
            if c:
                seg = order[starts[g] : starts[g] + c]
                gl = (idx[seg] - (g % GROUPS_PER_SEG) * TW_ROWS).astype(np.int16)
                g_local[o : o + c] = gl
                # duplicate-row tail pads: -1 trimming looks cheaper on paper
                # but crashes the device ucode; duplicate gathers are safe
                g_local[o + c : o + CAP] = gl[0]
                rank[seg] = o + np.arange(c)
            # all-pad / empty groups keep g_local 0 (valid row of the window
            # for tw<n_tw; tw==31 pad groups gather window 30 row 0 -- see
            # build: pad group uses base of last real window)
        # temp row of stream slot m (partition-major tile store):
        # t=m//TILE_TOK, b=(m%TILE_TOK)//P, p=m%P -> row t*TILE_TOK + p*(TILE_TOK//P) + b
        t_, r_ = rank // TILE_TOK, rank % TILE_TOK
        temp_row = t_ * TILE_TOK + (r_ % P) * (TILE_TOK // P) + r_ // P
        seg_local = temp_row - (pos // OW_ROWS) * self.seg_tok
        assert seg_local.min() >= 0 and seg_local.max() < self.seg_tok
        ridx = seg_local.astype(np.int16)

        def wrap(a, tokens_per_unit):
            a = a.reshape(-1, tokens_per_unit // 16, 16)
            a = np.swapaxes(a, 1, 2)
            return np.ascontiguousarray(np.tile(a, (1, 8, 1)))

        gidx = wrap(g_local, TILE_TOK)               # [n_tiles_a, 128, 1024]
        # pass B idx arrays per chunk (pad tail of last chunk with 0)
        rpad = np.zeros(self.n_chunks_b * TILE_TOK, dtype=np.int16)
        rpad[:n] = ridx
        return gidx, wrap(rpad, TILE_TOK)            # [n_chunks_b, 128, 1024]

    def extract(self, out_buf):
        # out_buf [n_chunks_b, P, TILE_TOK//P, d]: row c*TILE_TOK + b*P + p at [c, p, b]
        a = np.transpose(out_buf, (0, 2, 1, 3)).reshape(-1, self.d)
        return a[: self.per_core]


def _build_bass(plan, dt_name):
    import concourse.bacc as bacc
    import concourse.mybir as mybir
    import concourse.tile as tile

    key = (plan.num_emb, plan.d, plan.per_core, dt_name)
    if key in _CACHE:
        return _CACHE[key]

    dt = getattr(mybir.dt, dt_name)
    d = plan.d
    blk = TILE_TOK // P                              # 128 blocks per tile
    nc = bacc.Bacc(
        "TRN2", target_bir_lowering=False, debug=False, num_devices=N_CORES,
        num_swdge_queues=4,
    )
    weight = nc.dram_tensor("weight", [plan.num_emb, d], dt, kind="ExternalInput")
    gidx = nc.dram_tensor(
        "gidx", [plan.n_tiles_a, P, TILE_TOK // 16], mybir.dt.int16, kind="ExternalInput"
    )
    ridx = nc.dram_tensor(
        "ridx", [plan.n_chunks_b, P, TILE_TOK // 16], mybir.dt.int16, kind="ExternalInput"
    )
    # ExternalOutput: the bass2jax/axon path binds only External tensors.
    # One temp per segment: pass-B chunk c then only depends on the stores
    # of its own segment (Tile tracks DRAM deps per tensor), so B gathers
    # need not wait for the final A store.
    temps = [
        nc.dram_tensor(f"temp{s}", [plan.seg_tok, d], dt, kind="ExternalOutput")
        for s in range(plan.n_ow)
    ]
    outb = nc.dram_tensor(
        "outb", [plan.n_chunks_b, P, blk, d], dt, kind="ExternalOutput"
    )

    bpg = CAP // P                                   # blocks per group
    with tile.TileContext(nc) as tc:
        with (
            tc.tile_pool(name="gip", bufs=3) as gip,
            tc.tile_pool(name="data", bufs=3) as datap,
        ):
            q = 0
            # pass A: sorted windowed gathers -> temp (partition-major);
            # idx loads ride the scalar HWDGE ring so they never queue
            # behind the 4 MB data stores on the sync ring
            for t, gathers in enumerate(plan.tiles_a):
                git = gip.tile([P, TILE_TOK // 16], mybir.dt.int16)
                nc.scalar.dma_start(git[:], gidx[t, :, :])
                dtile = datap.tile([P, blk * d], dt)
                d3 = dtile[:].rearrange("p (b d) -> p b d", d=d)
                for tok_off, tw in gathers:
                    if tw >= plan.n_tw:
                        continue        # all-pad segment-alignment group
                    base = tw * TW_ROWS
                    rows = min(TW_ROWS, plan.num_emb - base)
                    nc.gpsimd.dma_gather(
                        out_ap=d3[:, tok_off // P : tok_off // P + bpg, :],
                        in_ap=weight[base : base + rows, :],
                        idxs_ap=git[:, tok_off // 16 : (tok_off + CAP) // 16],
                        num_idxs=CAP,
                        num_idxs_reg=CAP,
                        elem_size=d,
                        queue_num=q % 4,
                    )
                    q += 1
                s, h = divmod(t, 2)
                tv = temps[s][:].rearrange("(h p b) d -> h p (b d)", h=2, p=P)
                nc.sync.dma_start(tv[h, :, :], dtile[:])
            # pass B: rank re-gather per out chunk -> outb (partition-major)
            for c, (a0, ntok, runs) in enumerate(plan.chunks_b):
                rit = gip.tile([P, TILE_TOK // 16], mybir.dt.int16)
                nc.scalar.dma_start(rit[:], ridx[c, :, :])
                dtile = datap.tile([P, blk * d], dt)
                d3 = dtile[:].rearrange("p (b d) -> p b d", d=d)
                for tok_off, nt, ow in runs:
                    nc.gpsimd.dma_gather(
                        out_ap=d3[:, tok_off // P : (tok_off + nt) // P, :],
                        in_ap=temps[ow][:],
                        idxs_ap=rit[:, tok_off // 16 : (tok_off + nt) // 16],
                        num_idxs=nt,
                        num_idxs_reg=nt,
                        elem_size=d,
                        queue_num=q % 4,
                    )
                    q += 1
                nb = -(-ntok // P)
                nc.sync.dma_start(
                    outb[c, :, :nb, :].rearrange("p b d -> p (b d)"),
                    dtile[:, : nb * d],
                )
    nc.compile()
    _CACHE[key] = nc
    return nc


def run_sharded(indices: np.ndarray, weight: np.ndarray, trace: bool = False):
    from concourse.bass_utils import run_bass_kernel_spmd

    dt_np = ml_dtypes.bfloat16 if DT_NAME == "bfloat16" else np.float32
    idx_flat = np.ascontiguousarray(indices.reshape(-1).astype(np.int64))
    n_idx = idx_flat.shape[0]
    per_core = n_idx // N_CORES
    assert n_idx == per_core * N_CORES

    plan = Plan(per_core=per_core)
    nc = _build_bass(plan, DT_NAME)

    w = np.ascontiguousarray(weight.astype(dt_np))
    in_maps = []
    for c in range(N_CORES):
        gidx, ridx = plan.route(idx_flat[c * per_core : (c + 1) * per_core])
        in_maps.append({"weight": w, "gidx": gidx, "ridx": ridx})
    res = run_bass_kernel_spmd(
        nc, in_maps, core_ids=list(range(N_CORES)), trace=trace
    )
    full = np.concatenate(
        [plan.extract(r["outb"]) for r in res.results], axis=0
    ).astype(np.float32)
    return full.reshape(indices.shape + (D,)), res


def kernel(indices: np.ndarray, weight: np.ndarray) -> np.ndarray:
    full, _ = run_sharded(indices, weight, trace=False)
    return full


# revision 22
# speedup vs baseline: 1.0633x; 1.0633x over previous
"""Embedding lookup (weight[indices]) on 8 TRN2 NeuronCores.

Table replicated per core (bf16), indices sharded 8 ways.  Per-row
indirect DMA costs ~8.6 ns/row of serial Q7 descriptor generation, and
dma_scatter_add costs ~5.5 ns/token plus a CCE read-modify-write, so the
kernel uses a two-pass gather radix built only from dma_gather
(~2.3 ns/token, the cheapest data-dependent primitive):

  pass A: host sorts the core's tokens by (out_window, table_window)
          into fixed 1024-token groups (int16 window-local indices,
          duplicate-row pads); windowed dma_gathers fill SBUF tiles in
          stream order; contiguous HWDGE stores write the tiles to an
          HBM temp in partition-major layout (32 KB descriptors).
  pass B: each 32768-row temp segment holds exactly one out-window's
          tokens, so an int16 rank re-gathers them in original order;
          contiguous stores write a partition-major out buffer that the
          host transposes back (static, index-independent reshape).

Group capacity is fixed, so the instruction structure depends only on
shapes: one compiled NEFF serves all cores and calls.  bf16 halves HBM
traffic; its rounding (~0.4% rel err) is well inside the 2e-2 gate.
"""

import numpy as np
import ml_dtypes

P = 128
D = 128
NUM_EMB = 1_000_000
N_CORES = 8

TW_ROWS = 32768     # table window (int16 gather index range)
OW_ROWS = 25600     # out rows per segment (per_core / 4)
CAP = 1024          # fixed tokens per (ow, tw) group; avg ~826
GROUPS_PER_SEG = 32 # 31 real table windows + 1 all-pad group -> 32768 rows
TILE_TOK = 16384    # tokens per SBUF tile (16 groups)
DT_NAME = "bfloat16"

_CACHE = {}


class Plan:
    def __init__(self, num_emb=NUM_EMB, d=D, per_core=102400):
        self.num_emb, self.d, self.per_core = num_emb, d, per_core
        self.n_tw = -(-num_emb // TW_ROWS)           # 31
        assert self.n_tw < GROUPS_PER_SEG
        self.n_ow = per_core // OW_ROWS              # 4
        assert per_core == self.n_ow * OW_ROWS and OW_ROWS % P == 0
        self.seg_tok = GROUPS_PER_SEG * CAP          # 32768 temp rows/segment
        self.n_groups = self.n_ow * GROUPS_PER_SEG   # 128 (incl pad groups)
        self.groups_per_tile = TILE_TOK // CAP       # 16
        self.n_tiles_a = self.n_groups // self.groups_per_tile  # 8
        self.temp_rows = self.n_groups * CAP         # 131072
        # pass A static structure: per tile, gathers (tok_off, tw)
        self.tiles_a = []
        for t in range(self.n_tiles_a):
            g0 = t * self.groups_per_tile
            self.tiles_a.append(
                [((g - g0) * CAP, g % GROUPS_PER_SEG) for g in range(g0, g0 + self.groups_per_tile)]
            )
        # pass B: chunks of TILE_TOK out rows; per chunk, gather runs
        # (tok_off, ntok, ow) split at out-window boundaries
        self.chunks_b = []
        a = 0
        while a < per_core:
            n = min(TILE_TOK, per_core - a)
            runs = []
            o = a
            MAXTOK = 1024   # one gather stays within the SWDGE desc ring
                            # (4096-token gathers crash the device; 1024 is
                            # the empirically safe size, same as pass A's CAP)
            while o < a + n:
                ow = o // OW_ROWS
                e = min(a + n, (ow + 1) * OW_ROWS, o + MAXTOK)
                runs.append((o - a, e - o, ow))
                o = e
            self.chunks_b.append((a, n, runs))
            a += n
        self.n_chunks_b = len(self.chunks_b)

    def route(self, idx):
        n = self.per_core
        assert idx.shape == (n,)
        pos = np.arange(n, dtype=np.int64)
        ow = pos // OW_ROWS
        tw = idx // TW_ROWS
        key = ow * GROUPS_PER_SEG + tw               # pad group (tw=31) unused by real tokens
        order = np.argsort(key, kind="stable")
        counts = np.bincount(key[order], minlength=self.n_groups)
        if counts.max() > CAP:
            raise OverflowError(f"group count {counts.max()} > cap {CAP}")
        total = self.n_tiles_a * TILE_TOK
        g_local = np.zeros(total, dtype=np.int16)    # pass A window-local row
        rank = np.zeros(n, dtype=np.int64)           # token -> padded stream slot
        starts = np.concatenate([[0], np.cumsum(counts)[:-1]])
        for g in range(self.n_groups):
            c = counts[g]
            o = g * CAP
            if c:
                seg = order[starts[g] : starts[g] + c]
                gl = (idx[seg] - (g % GROUPS_PER_SEG) * TW_ROWS).astype(np.int16)
                g_local[o : o + c] = gl
                # duplicate-row tail pads: -1 trimming looks cheaper on paper
                # but crashes the device ucode; duplicate gathers are safe
                g_local[o + c : o + CAP] = gl[0]
                rank[seg] = o + np.arange(c)
            # all-pad / empty groups keep g_local 0 (valid row of the window
            # for tw<n_tw; tw==31 pad groups gather window 30 row 0 -- see
            # build: pad group uses base of last real window)
        # temp row of stream slot m (partition-major tile store):
        # t=m//TILE_TOK, b=(m%TILE_TOK)//P, p=m%P -> row t*TILE_TOK + p*(TILE_TOK//P) + b
        t_, r_ = rank // TILE_TOK, rank % TILE_TOK
        temp_row = t_ * TILE_TOK + (r_ % P) * (TILE_TOK // P) + r_ // P
        seg_local = temp_row - (pos // OW_ROWS) * self.seg_tok
        assert seg_local.min() >= 0 and seg_local.max() < self.seg_tok
        ridx = seg_local.astype(np.int16)

        def wrap(a, tokens_per_unit):
            a = a.reshape(-1, tokens_per_unit // 16, 16)
            a = np.swapaxes(a, 1, 2)
            return np.ascontiguousarray(np.tile(a, (1, 8, 1)))

        gidx = wrap(g_local, TILE_TOK)               # [n_tiles_a, 128, 1024]
        # pass B idx arrays per chunk (pad tail of last chunk with 0)
        rpad = np.zeros(self.n_chunks_b * TILE_TOK, dtype=np.int16)
        rpad[:n] = ridx
        return gidx, wrap(rpad, TILE_TOK)            # [n_chunks_b, 128, 1024]

    def extract(self, out_buf):
        # out_buf [n_chunks_b, P, TILE_TOK//P, d]: row c*TILE_TOK + b*P + p at [c, p, b]
        a = np.transpose(out_buf, (0, 2, 1, 3)).reshape(-1, self.d)
        return a[: self.per_core]


def _build_bass(plan, dt_name):
    import concourse.bacc as bacc
    import concourse.mybir as mybir
    import concourse.tile as tile

    key = (plan.num_emb, plan.d, plan.per_core, dt_name)
    if key in _CACHE:
        return _CACHE[key]

    dt = getattr(mybir.dt, dt_name)
    d = plan.d
    blk = TILE_TOK // P                              # 128 blocks per tile
    nc = bacc.Bacc(
        "TRN2", target_bir_lowering=False, debug=False, num_devices=N_CORES,
        num_swdge_queues=4,
    )
    weight = nc.dram_tensor("weight", [plan.num_emb, d], dt, kind="ExternalInput")
    gidx = nc.dram_tensor(
        "gidx", [plan.n_tiles_a, P, TILE_TOK // 16], mybir.dt.int16, kind="ExternalInput"
    )
    ridx = nc.dram_tensor(
        "ridx", [plan.n_chunks_b, P, TILE_TOK // 16], mybir.dt.int16, kind="ExternalInput"
    )
    # ExternalOutput: the bass2jax/axon path binds only External tensors
    temp = nc.dram_tensor("temp", [plan.temp_rows, d], dt, kind="ExternalOutput")
    outb = nc.dram_tensor(
        "outb", [plan.n_chunks_b, P, blk, d], dt, kind="ExternalOutput"
    )

    bpg = CAP // P                                   # blocks per group
    with tile.TileContext(nc) as tc:
        with (
            tc.tile_pool(name="gip", bufs=2) as gip,
            tc.tile_pool(name="data", bufs=3) as datap,
        ):
            q = 0
            # pass A: sorted windowed gathers -> temp (partition-major)
            temp4 = temp[:].rearrange("(t p b) d -> t p (b d)", t=plan.n_tiles_a, p=P)
            for t, gathers in enumerate(plan.tiles_a):
                git = gip.tile([P, TILE_TOK // 16], mybir.dt.int16)
                nc.sync.dma_start(git[:], gidx[t, :, :])
                dtile = datap.tile([P, blk * d], dt)
                d3 = dtile[:].rearrange("p (b d) -> p b d", d=d)
                for tok_off, tw in gathers:
                    if tw >= plan.n_tw:
                        continue        # all-pad segment-alignment group
                    base = tw * TW_ROWS
                    rows = min(TW_ROWS, plan.num_emb - base)
                    nc.gpsimd.dma_gather(
                        out_ap=d3[:, tok_off // P : tok_off // P + bpg, :],
                        in_ap=weight[base : base + rows, :],
                        idxs_ap=git[:, tok_off // 16 : (tok_off + CAP) // 16],
                        num_idxs=CAP,
                        num_idxs_reg=CAP,
                        elem_size=d,
                        queue_num=q % 4,
                    )
                    q += 1
                nc.sync.dma_start(temp4[t, :, :], dtile[:])
            # pass B: rank re-gather per out chunk -> outb (partition-major)
            for c, (a0, ntok, runs) in enumerate(plan.chunks_b):
                rit = gip.tile([P, TILE_TOK // 16], mybir.dt.int16)
                nc.sync.dma_start(rit[:], ridx[c, :, :])
                dtile = datap.tile([P, blk * d], dt)
                d3 = dtile[:].rearrange("p (b d) -> p b d", d=d)
                for tok_off, nt, ow in runs:
                    nc.gpsimd.dma_gather(
                        out_ap=d3[:, tok_off // P : (tok_off + nt) // P, :],
                        in_ap=temp[ow * plan.seg_tok : (ow + 1) * plan.seg_tok, :],
                        idxs_ap=rit[:, tok_off // 16 : (tok_off + nt) // 16],
                        num_idxs=nt,
                        num_idxs_reg=nt,
                        elem_size=d,
                        queue_num=q % 4,
                    )
                    q += 1
                nb = -(-ntok // P)
                nc.sync.dma_start(
                    outb[c, :, :nb, :].rearrange("p b d -> p (b d)"),
                    dtile[:, : nb * d],
                )
    nc.compile()
    _CACHE[key] = nc
    return nc


def run_sharded(indices: np.ndarray, weight: np.ndarray, trace: bool = False):
    from concourse.bass_utils import run_bass_kernel_spmd

    dt_np = ml_dtypes.bfloat16 if DT_NAME == "bfloat16" else np.float32
    idx_flat = np.ascontiguousarray(indices.reshape(-1).astype(np.int64))
    n_idx = idx_flat.shape[0]
    per_core = n_idx // N_CORES
    assert n_idx == per_core * N_CORES

    plan = Plan(per_core=per_core)
    nc = _build_bass(plan, DT_NAME)

    w = np.ascontiguousarray(weight.astype(dt_np))
    in_maps = []
    for c in range(N_CORES):
        gidx, ridx = plan.route(idx_flat[c * per_core : (c + 1) * per_core])
        in_maps.append({"weight": w, "gidx": gidx, "ridx": ridx})
    res = run_bass_kernel_spmd(
        nc, in_maps, core_ids=list(range(N_CORES)), trace=trace
    )
    full = np.concatenate(
        [plan.extract(r["outb"]) for r in res.results], axis=0
    ).astype(np.float32)
    return full.reshape(indices.shape + (D,)), res


def kernel(indices: np.ndarray, weight: np.ndarray) -> np.ndarray:
    full, _ = run_sharded(indices, weight, trace=False)
    return full
